# revision 11
# baseline (speedup 1.0000x reference)
"""DriftAwareLightMemory fused Bass/Tile kernel for 8 trn2 NeuronCores.

Strategy (L-sharded data parallel):
  - Shard the sequence axis L=1024 into 8 chunks of 128 rows; each core gets
    x[:, k*128:(k+1)*128] and memory_snapshot[:, :, k*128:(k+1)*128].
  - Each core keeps its 16 MB memory chunk resident in SBUF, computes
    per-(b,t) column sums over its L rows (for the means), the per-row
    DriftCorrectionExtractor matmuls, and partial sums for q_global/cur_drift.
  - One 147 KB 8-core AllReduce combines the partials; every core then
    redundantly computes the tiny time-attention softmax and finishes its
    L-chunk: enhanced = sum_t attn[b,t]*memory[b,t] via PSUM-accumulated diag
    matmuls over the SBUF-resident chunk, then the fuse gate and output.
  - Matmuls run as float32r (fp32 with 12-bit significand): full PE speed,
    ~1.2e-4 input rounding; predicted end-to-end absmax rel err ~1.5e-4.

kernel(**inputs) takes full-size numpy inputs, returns [4,1024,512] float32.
"""
import sys
import math

sys.path.insert(0, "/opt/trn_rl_repo")

import numpy as np

import concourse.bass as bass
import concourse.bacc as bacc
import concourse.tile as tile
from concourse import bass_utils, mybir

dt = mybir.dt
AF = mybir.ActivationFunctionType
ALU = mybir.AluOpType

B, T, L, D = 4, 16, 1024, 512
NC = 8
LC = L // NC            # 128 L rows per core
ROWS = B * LC           # 512 fm rows per core (row = b*128 + l)
NCH = D // 128          # 4 feature-partition chunks
LAMBDA = 0.3
C_CONT = 1.0 / math.sqrt(D)
C_DRIFT = -LAMBDA / D
INV_L = 1.0 / L

_CACHE = {}


def _round_f32r(x):
    """Round fp32 to the FP32R grid (12-bit significand, RNE)."""
    x = np.ascontiguousarray(x, dtype=np.float32)
    b = x.view(np.uint32)
    lsb = (b >> np.uint32(12)) & np.uint32(1)
    out = (b + np.uint32(0x7FF) + lsb) & np.uint32(0xFFFFF000)
    return out.view(np.float32)


def _wdev(w):
    """[512,512] weight -> [128,2048] device layout (k-chunk c at cols c*512)."""
    return np.ascontiguousarray(
        w.reshape(4, 128, 512).transpose(1, 0, 2).reshape(128, 2048))


def _bias_fm(b):
    return np.ascontiguousarray(b.reshape(4, 128).T)


def _sin_table():
    pos = np.arange(1, T + 1, dtype=np.float32)
    half = D // 2
    div = np.exp(-math.log(10000.0) * (2.0 * np.arange(half, dtype=np.float32) / D))
    ang = pos[:, None] * div
    pe = np.stack([np.sin(ang), np.cos(ang)], axis=-1).reshape(T, D)
    return pe.astype(np.float32)


def _build():
    nc = bacc.Bacc("TRN2", target_bir_lowering=False, debug=False, num_devices=NC)
    f32, f32r = dt.float32, dt.float32r

    MEM = nc.dram_tensor("MEM", [B, T, LC, D], f32r, kind="ExternalInput").ap()
    XPH = nc.dram_tensor("XPH", [B, LC, D], f32r, kind="ExternalInput").ap()
    XK = nc.dram_tensor("XK", [B, LC, D], f32r, kind="ExternalInput").ap()
    WR_names = ["wd", "wx", "wpn", "gx", "gp", "wo", "f1", "f2", "seqw"]
    WR = {n: nc.dram_tensor("W_" + n, [128, 2048], f32r, kind="ExternalInput").ap()
          for n in WR_names}
    WF_names = ["wm", "wmd", "wq", "wcd"]
    WF = {n: nc.dram_tensor("W_" + n, [128, 2048], f32, kind="ExternalInput").ap()
          for n in WF_names}
    BIAS = nc.dram_tensor("BIAS", [128, 36], f32, kind="ExternalInput").ap()
    SEQB = nc.dram_tensor("SEQB", [1, 512], f32r, kind="ExternalInput").ap()
    SINT = nc.dram_tensor("SINT", [128, 64], f32r, kind="ExternalInput").ap()
    IDENT = nc.dram_tensor("IDENT", [128, 128], f32, kind="ExternalInput").ap()
    IDENTR = nc.dram_tensor("IDENTR", [128, 128], f32r, kind="ExternalInput").ap()
    ONESC = nc.dram_tensor("ONESC", [128, 1], f32r, kind="ExternalInput").ap()
    ONESR = nc.dram_tensor("ONESR", [1, 128], f32r, kind="ExternalInput").ap()
    IDENTN = nc.dram_tensor("IDENTN", [128, 128], f32r, kind="ExternalInput").ap()
    OUT = nc.dram_tensor("OUT", [B, LC, D], f32, kind="ExternalOutput").ap()

    BI = {n: i for i, n in enumerate(
        ["b_A", "b_t1", "gate_b", "outp_b", "q_b", "mem_b", "curd_b",
         "memd_b", "fuse_b"])}

    with tile.TileContext(nc) as tc:
        with tc.tile_pool(name="sb", bufs=1) as sb, \
             tc.tile_pool(name="ps", bufs=1, space="PSUM") as ps, \
             tc.tile_pool(name="dram", bufs=1, space="DRAM") as dram:

            def S(shape, dtype, tag, bufs=1):
                return sb.tile(shape, dtype, tag=tag, bufs=bufs, name=tag)

            def P(shape, tag, bufs=1, dtype=dt.float32):
                return ps.tile(shape, dtype, tag=tag, bufs=bufs, name=tag)

            # ---------------- constants ----------------
            ident = S([128, 128], f32, "ident")
            identr = S([128, 128], f32r, "identr")
            onesc = S([128, 1], f32r, "onesc")
            onesr = S([1, 128], f32r, "onesr")
            biases = S([128, 36], f32, "biases")
            seqb = S([1, 512], f32r, "seqb")
            sint = S([128, 64], f32r, "sint")
            identn = S([128, 128], f32r, "identn")
            nc.sync.dma_start(ident, IDENT)
            nc.sync.dma_start(identr, IDENTR)
            nc.sync.dma_start(onesc, ONESC)
            nc.sync.dma_start(onesr, ONESR)
            nc.sync.dma_start(biases, BIAS)
            nc.sync.dma_start(seqb, SEQB)
            nc.sync.dma_start(sint, SINT)
            nc.sync.dma_start(identn, IDENTN)

            def bias_col(name):
                return biases[:, BI[name]:BI[name] + 1]

            # ---------------- input loads ----------------
            x_nat, xp_nat = [], []
            for b in range(B):
                xt = S([128, 512], f32r, "xnat", bufs=1)
                nc.sync.dma_start(xt, XK[b])
                x_nat.append(xt)
                pt_ = S([128, 512], f32r, "xpnat", bufs=1)
                nc.sync.dma_start(pt_, XPH[b])
                xp_nat.append(pt_)

            mem_res = {}

            def load_mem(b, h):
                tag = f"m{b}h{h}" if b < 2 else f"m{b}s"
                mt = S([128, 8 * 512], f32r, tag)
                nc.sync.dma_start(
                    mt, MEM[b, 8 * h:8 * (h + 1)].rearrange("t l d -> l t d"))
                mem_res[(b, h)] = mt

            for b in range(2):
                for h in range(2):
                    load_mem(b, h)

            def mem_slice(b, t):
                return mem_res[(b, t // 8)][:, (t % 8) * 512:(t % 8 + 1) * 512]

            # ---------------- fm transposes ----------------
            x_fm = [S([128, ROWS], f32r, "xfm", bufs=4) for _ in range(NCH)]
            for b in range(B):
                for c in range(NCH):
                    pt = P([128, 128], "pt", bufs=2, dtype=f32r)
                    nc.tensor.transpose(pt, x_nat[b][:, c * 128:(c + 1) * 128],
                                        identr)
                    nc.scalar.copy(x_fm[c][:, b * 128:(b + 1) * 128], pt)
            xp_fm = [S([128, ROWS], f32r, "quadA", bufs=4) for _ in range(NCH)]
            for b in range(B):
                for c in range(NCH):
                    pt = P([128, 128], "pt", bufs=2, dtype=f32r)
                    nc.tensor.transpose(pt, xp_nat[b][:, c * 128:(c + 1) * 128],
                                        identr)
                    nc.vector.tensor_copy(xp_fm[c][:, b * 128:(b + 1) * 128], pt)

            ar_in = dram.tile([72, 512], f32, tag="ar_in", name="ar_in")
            ar_out = dram.tile([72, 512], f32, tag="ar_out", name="ar_out")

            # ---------------- L-sum colsums ----------------
            def emit_lsum(b, t):
                psum = P([1, 512], "pcs", bufs=2)
                nc.tensor.matmul(psum, onesc, mem_slice(b, t), start=True,
                                 stop=True)
                st = S([1, 512], f32, "stage", bufs=2)
                if (b * T + t) % 2 == 0:
                    nc.scalar.copy(st, psum)
                else:
                    nc.vector.tensor_copy(st, psum)
                bt = b * T + t
                nc.sync.dma_start(ar_in[bt:bt + 1, :], st)

            for t in range(8):
                emit_lsum(0, t)

            # ---------------- weight groups ----------------
            def load_w(name, table=WR, dtype=f32r):
                ta = sb.tile([128, 1024], dtype, tag="w", bufs=4, name="wa_" + name)
                tb = sb.tile([128, 1024], dtype, tag="w", bufs=4, name="wb_" + name)
                nc.sync.dma_start(ta, table[name][:, 0:1024])
                nc.sync.dma_start(tb, table[name][:, 1024:2048])
                return (ta, tb)

            def w_chunk(wt, c_k, c_out):
                half = wt[c_k // 2]
                off = (c_k % 2) * 512 + c_out * 128
                return half[:, off:off + 128]

            def mm_group(pairs, out_maker, n=ROWS):
                for c_out in range(NCH):
                    psum = P([128, n], "pmm", bufs=2)
                    first = True
                    for pi, (wt, rhs_list) in enumerate(pairs):
                        for c_k in range(NCH):
                            last = (pi == len(pairs) - 1) and (c_k == NCH - 1)
                            nc.tensor.matmul(
                                psum, w_chunk(wt, c_k, c_out), rhs_list[c_k],
                                start=first, stop=last)
                            first = False
                    out_maker(c_out, psum)

            # delta
            delta_fm = []
            for c in range(NCH):
                dfm = S([128, ROWS], f32r, "quadB", bufs=4)
                nc.vector.tensor_tensor(out=dfm, in0=x_fm[c], in1=xp_fm[c],
                                        op=ALU.subtract)
                delta_fm.append(dfm)

            dsum = [S([128, 4], f32, f"dsum{c}") for c in range(NCH)]
            for c in range(NCH):
                for b in range(B):
                    nc.vector.reduce_sum(
                        out=dsum[c][:, b:b + 1],
                        in_=delta_fm[c][:, b * 128:(b + 1) * 128],
                        axis=mybir.AxisListType.X)

            wx = load_w("wx")
            wpn = load_w("wpn")
            t1 = [None] * NCH

            def mk_t1(c, psum):
                o = S([128, ROWS], f32r, "feat", bufs=10)
                nc.scalar.activation(o, psum, AF.Identity, bias=bias_col("b_t1"))
                t1[c] = o
            mm_group([(wx, x_fm), (wpn, xp_fm)], mk_t1)

            for t in range(8, 16):
                emit_lsum(0, t)

            # A = delta@Wd - t1 + (delta_b - b_t1); then mid folds in place
            wd = load_w("wd")
            afeat = [None] * NCH
            for c_out in range(NCH):
                psum = P([128, ROWS], "pmm", bufs=2)
                for c_k in range(NCH):
                    nc.tensor.matmul(psum, w_chunk(wd, c_k, c_out),
                                     delta_fm[c_k], start=(c_k == 0), stop=False)
                nc.tensor.matmul(psum, identn, t1[c_out], start=False, stop=True)
                o = S([128, ROWS], f32r, "feat", bufs=10)
                nc.scalar.activation(o, psum, AF.Identity, bias=bias_col("b_A"))
                afeat[c_out] = o

            for t in range(8):
                emit_lsum(1, t)

            # phase A for b=3 memory: L-sums only (streamed via single slot)
            load_mem(3, 0)

            gxw = load_w("gx")
            gpw = load_w("gp")
            gsig = [None] * NCH

            def mk_g(c, psum):
                o = S([128, ROWS], f32r, "feat", bufs=10)
                nc.scalar.activation(o, psum, AF.Sigmoid, bias=bias_col("gate_b"))
                gsig[c] = o
            mm_group([(gxw, x_fm), (gpw, xp_fm)], mk_g)

            for t in range(8, 16):
                emit_lsum(1, t)
            load_mem(2, 0)
            for t in range(8):
                emit_lsum(2, t)

            # mid = t1 + g*A  (in place in afeat)
            mid = afeat
            for c in range(NCH):
                nc.vector.tensor_tensor(out=afeat[c], in0=afeat[c], in1=gsig[c],
                                        op=ALU.mult)
                nc.vector.tensor_tensor(out=afeat[c], in0=afeat[c], in1=t1[c],
                                        op=ALU.add)

            load_mem(2, 1)
            for t in range(8, 16):
                emit_lsum(2, t)
            for t in range(8):
                emit_lsum(3, t)
            load_mem(3, 1)
            for t in range(8, 16):
                emit_lsum(3, t)

            wo = load_w("wo")
            raw_fm = [None] * NCH

            def mk_raw(c, psum):
                o = S([128, ROWS], f32, "raw", bufs=4)
                nc.scalar.activation(o, psum, AF.Identity, bias=bias_col("outp_b"))
                raw_fm[c] = o
            mm_group([(wo, mid)], mk_raw)

            qsum = [S([128, 4], f32, f"qsum{c}") for c in range(NCH)]
            for c in range(NCH):
                xr = S([128, ROWS], f32, "feat", bufs=10)
                nc.vector.tensor_tensor(out=xr, in0=x_fm[c], in1=raw_fm[c],
                                        op=ALU.add)
                for b in range(B):
                    nc.vector.reduce_sum(
                        out=qsum[c][:, b:b + 1],
                        in_=xr[:, b * 128:(b + 1) * 128],
                        axis=mybir.AxisListType.X)

            qn = S([4, 512], f32, "qn")
            dn = S([4, 512], f32, "dn")
            for c in range(NCH):
                pt = P([128, 128], "pt", bufs=2)
                nc.tensor.transpose(pt[0:4, :], qsum[c], ident)
                nc.scalar.copy(qn[:, c * 128:(c + 1) * 128], pt[0:4, :])
                pt2 = P([128, 128], "pt", bufs=2)
                nc.tensor.transpose(pt2[0:4, :], dsum[c], ident)
                nc.scalar.copy(dn[:, c * 128:(c + 1) * 128], pt2[0:4, :])
            nc.sync.dma_start(ar_in[64:68, :], qn)
            nc.sync.dma_start(ar_in[68:72, :], dn)

            # ---------------- AllReduce ----------------
            nc.gpsimd.collective_compute(
                "AllReduce", ALU.add,
                replica_groups=[list(range(NC))],
                ins=[ar_in[:]], outs=[ar_out[:]])

            f1w = load_w("f1")
            f1log = [None] * NCH

            def mk_f1(c, psum):
                o = S([128, ROWS], f32, "feat", bufs=10)
                nc.vector.tensor_copy(o, psum)
                f1log[c] = o
            mm_group([(f1w, x_fm)], mk_f1)

            S_m = S([64, 512], f32, "S_m")
            qrows = S([4, 512], f32, "qrows")
            drows = S([4, 512], f32, "drows")
            nc.sync.dma_start(S_m, ar_out[0:64, :])
            nc.sync.dma_start(qrows, ar_out[64:68, :])
            nc.sync.dma_start(drows, ar_out[68:72, :])

            # pos_emb natural [16,512] (f32r)
            seqw = load_w("seqw")
            wm = load_w("wm", WF, f32)
            pe_psum = P([16, 512], "pmm", bufs=2)
            for c_k in range(NCH):
                nc.tensor.matmul(pe_psum, sint[:, c_k * 16:(c_k + 1) * 16],
                                 seqw[c_k // 2][:, (c_k % 2) * 512:
                                                (c_k % 2) * 512 + 512],
                                 start=(c_k == 0), stop=False)
            nc.tensor.matmul(pe_psum, onesr[:, 0:16], seqb, start=False, stop=True)
            pe_nat = S([16, 512], f32r, "pe_nat")
            nc.scalar.copy(pe_nat, pe_psum)

            # mean_fm + pos broadcast; md_fm diffs
            mean_fm, md_fm = [], []
            for c in range(NCH):
                pt = P([128, 128], "pt", bufs=2)
                nc.tensor.transpose(pt[:, 0:64], S_m[:, c * 128:(c + 1) * 128],
                                    ident[0:64, 0:64])
                mf = S([128, 64], f32, f"meanfm{c}")
                nc.scalar.activation(mf, pt[:, 0:64], AF.Copy, scale=INV_L)
                pt2 = P([128, 128], "pt", bufs=2)
                nc.tensor.transpose(pt2[:, 0:16],
                                    pe_nat.bitcast(f32)[:, c * 128:(c + 1) * 128],
                                    ident[0:16, 0:16])
                pf = S([128, 16], f32, f"posfm{c}")
                nc.vector.tensor_copy(pf, pt2[:, 0:16])
                bc = bass.AP(tensor=pf.tensor, offset=pf.offset,
                             ap=[pf.ap[0], [0, 4], [1, 16]])
                nc.vector.tensor_tensor(
                    out=mf.rearrange("p (b t) -> p b t", b=4),
                    in0=mf.rearrange("p (b t) -> p b t", b=4),
                    in1=bc, op=ALU.add)
                mean_fm.append(mf)
                md = S([128, 64], f32, f"mdfm{c}")
                for b in range(B):
                    o = b * 16
                    nc.vector.tensor_copy(md[:, o:o + 1], mf[:, o:o + 1])
                    nc.vector.tensor_tensor(
                        out=md[:, o + 1:o + 16],
                        in0=mf[:, o + 1:o + 16],
                        in1=mf[:, o:o + 15], op=ALU.subtract)
                md_fm.append(md)

            def small_group(wt, rhs_list, bias_name, n):
                outs = []
                for c_out in range(NCH):
                    psum = P([128, n], "pmm", bufs=2)
                    for c_k in range(NCH):
                        nc.tensor.matmul(
                            psum, w_chunk(wt, c_k, c_out), rhs_list[c_k],
                            start=(c_k == 0), stop=(c_k == NCH - 1))
                    o = S([128, n], f32, f"sg_{bias_name}{c_out}")
                    nc.scalar.activation(o, psum, AF.Identity, bias=bias_col(bias_name))
                    outs.append(o)
                return outs

            gm = small_group(wm, mean_fm, "mem_b", 64)
            wmd = load_w("wmd", WF, f32)
            dm = small_group(wmd, md_fm, "memd_b", 64)

            qin, cin = [], []
            for c in range(NCH):
                pt = P([128, 128], "pt", bufs=2)
                nc.tensor.transpose(pt[:, 0:4], qrows[:, c * 128:(c + 1) * 128],
                                    ident[0:4, 0:4])
                qi = S([128, 4], f32, f"qin{c}")
                nc.scalar.activation(qi, pt[:, 0:4], AF.Copy, scale=INV_L)
                qin.append(qi)
                pt2 = P([128, 128], "pt", bufs=2)
                nc.tensor.transpose(pt2[:, 0:4], drows[:, c * 128:(c + 1) * 128],
                                    ident[0:4, 0:4])
                ci = S([128, 4], f32, f"cin{c}")
                nc.scalar.activation(ci, pt2[:, 0:4], AF.Copy, scale=INV_L)
                cin.append(ci)
            wq = load_w("wq", WF, f32)
            qg = small_group(wq, qin, "q_b", 4)
            wcd = load_w("wcd", WF, f32)
            cd = small_group(wcd, cin, "curd_b", 4)

            # scores
            cont_ps = P([1, 64], "pcs", bufs=2)
            for c in range(NCH):
                pr = S([128, 64], f32r, "sc64", bufs=2)
                qb = bass.AP(tensor=qg[c].tensor, offset=qg[c].offset,
                             ap=[qg[c].ap[0], [1, 4], [0, 16]])
                nc.vector.tensor_tensor(
                    out=pr.rearrange("p (b t) -> p b t", b=4),
                    in0=gm[c].rearrange("p (b t) -> p b t", b=4),
                    in1=qb, op=ALU.mult)
                nc.tensor.matmul(cont_ps, onesc, pr, start=(c == 0),
                                 stop=(c == NCH - 1))
            sq_ps = P([1, 64], "pcs", bufs=2)
            for c in range(NCH):
                dd = S([128, 64], f32, "sc64", bufs=2)
                cb = bass.AP(tensor=cd[c].tensor, offset=cd[c].offset,
                             ap=[cd[c].ap[0], [1, 4], [0, 16]])
                nc.vector.tensor_tensor(
                    out=dd.rearrange("p (b t) -> p b t", b=4),
                    in0=dm[c].rearrange("p (b t) -> p b t", b=4),
                    in1=cb, op=ALU.subtract)
                sq = S([128, 64], f32r, "sc64", bufs=2)
                nc.scalar.activation(sq, dd, AF.Square)
                nc.tensor.matmul(sq_ps, onesc, sq, start=(c == 0),
                                 stop=(c == NCH - 1))

            score = S([1, 64], f32, "score")
            tmp_s = S([1, 64], f32, "tmp_s")
            nc.vector.tensor_scalar_mul(score, cont_ps, C_CONT)
            nc.vector.tensor_scalar_mul(tmp_s, sq_ps, C_DRIFT)
            nc.vector.tensor_tensor(out=score, in0=score, in1=tmp_s, op=ALU.add)
            mx = S([1, 4], f32, "mx")
            nc.vector.reduce_max(out=mx,
                                 in_=score.rearrange("p (b t) -> p b t", b=4),
                                 axis=mybir.AxisListType.X)
            mxb = bass.AP(tensor=mx.tensor, offset=mx.offset,
                          ap=[mx.ap[0], [1, 4], [0, 16]])
            sc2 = S([1, 64], f32, "sc2")
            nc.vector.tensor_tensor(out=sc2.rearrange("p (b t) -> p b t", b=4),
                                    in0=score.rearrange("p (b t) -> p b t", b=4),
                                    in1=mxb, op=ALU.subtract)
            ex = S([1, 64], f32, "ex")
            nc.scalar.activation(ex, sc2, AF.Exp)
            sm = S([1, 4], f32, "sm")
            nc.vector.reduce_sum(out=sm, in_=ex.rearrange("p (b t) -> p b t", b=4),
                                 axis=mybir.AxisListType.X)
            rs = S([1, 4], f32, "rs")
            nc.vector.reciprocal(rs, sm)
            rsb = bass.AP(tensor=rs.tensor, offset=rs.offset,
                          ap=[rs.ap[0], [1, 4], [0, 16]])
            attn = S([1, 64], f32r, "attn")
            nc.vector.tensor_tensor(out=attn.rearrange("p (b t) -> p b t", b=4),
                                    in0=ex.rearrange("p (b t) -> p b t", b=4),
                                    in1=rsb, op=ALU.mult)

            attn_dr = dram.tile([1, 64], f32r, tag="attn_dr", name="attn_dr")
            nc.sync.dma_start(attn_dr, attn)
            attn_t4 = S([16, 4], f32r, "attn_t4")
            rd = bass.AP(tensor=attn_dr.tensor, offset=attn_dr.offset,
                         ap=[[1, 16], [16, 4]])
            nc.sync.dma_start(attn_t4, rd)

            ab_ps = P([128, 64], "pmm", bufs=2)
            nc.tensor.matmul(ab_ps, onesr, attn, start=True, stop=True)
            ab = S([128, 64], f32, "ab")
            nc.vector.tensor_copy(ab, ab_ps)

            # ---------------- enhanced ----------------
            # b=2/b=3 memory tiles were released after their L-sums; reload
            load_mem(2, 0)
            load_mem(2, 1)
            load_mem(3, 0)
            load_mem(3, 1)
            enh_nat = []
            for b in range(B):
                ep = P([128, 512], "penh", bufs=2)
                for t in range(T):
                    dg = S([128, 128], f32r, "diag", bufs=2)
                    nc.vector.tensor_scalar_mul(dg, ident,
                                                ab[:, b * T + t:b * T + t + 1])
                    nc.tensor.matmul(ep, dg, mem_slice(b, t),
                                     start=(t == 0), stop=False)
                pc_ps = P([1, 512], "pcs", bufs=2)
                nc.tensor.matmul(pc_ps, attn_t4[:, b:b + 1], pe_nat,
                                 start=True, stop=True)
                pc_sb = S([1, 512], f32r, "stage2", bufs=1)
                nc.scalar.copy(pc_sb, pc_ps)
                nc.tensor.matmul(ep, onesr, pc_sb, start=False, stop=True)
                en = S([128, 512], f32, "quadA", bufs=4)
                nc.vector.tensor_copy(en, ep)
                enh_nat.append(en)

            enh_fm = [S([128, ROWS], f32r, "quadB", bufs=4) for _ in range(NCH)]
            for b in range(B):
                for c in range(NCH):
                    pt = P([128, 128], "pt", bufs=2)
                    nc.tensor.transpose(pt, enh_nat[b][:, c * 128:(c + 1) * 128],
                                        ident)
                    nc.scalar.copy(enh_fm[c][:, b * 128:(b + 1) * 128], pt)

            # fuse + output
            f2w = load_w("f2")
            for c_out in range(NCH):
                psum = P([128, ROWS], "pmm", bufs=2)
                for c_k in range(NCH):
                    nc.tensor.matmul(
                        psum, w_chunk(f2w, c_k, c_out), enh_fm[c_k],
                        start=(c_k == 0), stop=(c_k == NCH - 1))
                fl = S([128, ROWS], f32, "feat", bufs=10)
                nc.vector.tensor_tensor(out=fl, in0=psum, in1=f1log[c_out],
                                        op=ALU.add)
                fg = S([128, ROWS], f32, "feat", bufs=10)
                nc.scalar.activation(fg, fl, AF.Sigmoid, bias=bias_col("fuse_b"))
                prod = S([128, ROWS], f32, "feat", bufs=10)
                nc.vector.tensor_tensor(out=prod, in0=fg, in1=enh_fm[c_out],
                                        op=ALU.mult)
                s1 = S([128, ROWS], f32, "feat", bufs=10)
                nc.vector.tensor_tensor(out=s1, in0=prod, in1=raw_fm[c_out],
                                        op=ALU.add)
                ofm = S([128, ROWS], f32, "feat", bufs=10)
                nc.vector.tensor_tensor(out=ofm, in0=s1, in1=x_fm[c_out],
                                        op=ALU.add)
                for b in range(B):
                    pt = P([128, 128], "pt", bufs=2)
                    nc.tensor.transpose(pt, ofm[:, b * 128:(b + 1) * 128], ident)
                    on = S([128, 128], f32, "onat", bufs=2)
                    nc.vector.tensor_copy(on, pt)
                    nc.sync.dma_start(OUT[b][:, c_out * 128:(c_out + 1) * 128], on)

    nc.compile()
    return nc


def _prep_maps(inputs):
    x = np.ascontiguousarray(inputs["x"], dtype=np.float32)
    mem = np.ascontiguousarray(inputs["memory_snapshot"], dtype=np.float32)

    gw = np.asarray(inputs["gate_W"], np.float32)
    fw = np.asarray(inputs["fuse_W"], np.float32)
    r = _round_f32r
    weights_r = {
        "wd": r(np.asarray(inputs["delta_W"], np.float32)),
        "wx": r(np.asarray(inputs["xproj_W"], np.float32)),
        "wpn": r(-np.asarray(inputs["phys_W"], np.float32)),
        "gx": r(gw[0:512] + gw[512:1024]),
        "gp": r(gw[1024:1536] - gw[0:512]),
        "wo": r(np.asarray(inputs["outp_W"], np.float32)),
        "f1": r(fw[0:512]),
        "f2": r(fw[512:1024]),
        "seqw": r(np.asarray(inputs["seq_W"], np.float32)),
    }
    weights_f = {
        "wm": np.asarray(inputs["mem_W"], np.float32),
        "wmd": np.asarray(inputs["memd_W"], np.float32),
        "wq": np.asarray(inputs["q_W"], np.float32),
        "wcd": np.asarray(inputs["curd_W"], np.float32),
    }
    b_t1_v = (np.asarray(inputs["xproj_b"], np.float32)
              - np.asarray(inputs["phys_b"], np.float32))
    bias_mat = np.stack([
        _bias_fm(np.asarray(inputs["delta_b"], np.float32) - b_t1_v),
        _bias_fm(b_t1_v),
        _bias_fm(np.asarray(inputs["gate_b"], np.float32)),
        _bias_fm(np.asarray(inputs["outp_b"], np.float32)),
        _bias_fm(np.asarray(inputs["q_b"], np.float32)),
        _bias_fm(np.asarray(inputs["mem_b"], np.float32)),
        _bias_fm(np.asarray(inputs["curd_b"], np.float32)),
        _bias_fm(np.asarray(inputs["memd_b"], np.float32)),
        _bias_fm(np.asarray(inputs["fuse_b"], np.float32)),
    ], axis=1).reshape(128, 36)

    sin_t = _sin_table()
    sint_dev = np.zeros((128, 64), np.float32)
    for c in range(4):
        sint_dev[:, c * 16:(c + 1) * 16] = sin_t[:, c * 128:(c + 1) * 128].T

    shared = {("W_" + n): _wdev(w) for n, w in weights_r.items()}
    shared.update({("W_" + n): _wdev(w) for n, w in weights_f.items()})
    shared.update({
        "BIAS": np.ascontiguousarray(bias_mat),
        "SEQB": r(np.asarray(inputs["seq_b"], np.float32)).reshape(1, 512),
        "SINT": r(sint_dev),
        "IDENT": np.eye(128, dtype=np.float32),
        "IDENTR": np.eye(128, dtype=np.float32),
        "ONESC": np.ones((128, 1), np.float32),
        "ONESR": np.ones((1, 128), np.float32),
        "IDENTN": -np.eye(128, dtype=np.float32),
    })

    mem_r = _round_f32r(mem)
    x_r = _round_f32r(x)
    in_maps = []
    for k in range(NC):
        sl = slice(k * LC, (k + 1) * LC)
        m = dict(shared)
        m["MEM"] = np.ascontiguousarray(mem_r[:, :, sl, :])
        m["XPH"] = np.ascontiguousarray(mem_r[:, T - 1, sl, :])
        m["XK"] = np.ascontiguousarray(x_r[:, sl, :])
        in_maps.append(m)
    return in_maps


def kernel(**inputs):
    if "nc" not in _CACHE:
        _CACHE["nc"] = _build()
    ncb = _CACHE["nc"]
    in_maps = _prep_maps(inputs)
    res = bass_utils.run_bass_kernel_spmd(ncb, in_maps, core_ids=list(range(NC)))
    out = np.empty((B, L, D), np.float32)
    for k in range(NC):
        out[:, k * LC:(k + 1) * LC, :] = res.results[k]["OUT"]
    return out


# revision 18
# speedup vs baseline: 338.6175x; 338.6175x over previous
"""DriftAwareLightMemory fused Bass/Tile kernel for 8 trn2 NeuronCores.

Strategy (L-sharded data parallel):
  - Shard the sequence axis L=1024 into 8 chunks of 128 rows; each core gets
    x[:, k*128:(k+1)*128] and memory_snapshot[:, :, k*128:(k+1)*128].
  - Each core keeps its 16 MB memory chunk resident in SBUF, computes
    per-(b,t) column sums over its L rows (for the means), the per-row
    DriftCorrectionExtractor matmuls, and partial sums for q_global/cur_drift.
  - One 147 KB 8-core AllReduce combines the partials; every core then
    redundantly computes the tiny time-attention softmax and finishes its
    L-chunk: enhanced = sum_t attn[b,t]*memory[b,t] via PSUM-accumulated diag
    matmuls over the SBUF-resident chunk, then the fuse gate and output.
  - Matmuls run as float32r (fp32 with 12-bit significand): full PE speed,
    ~1.2e-4 input rounding; predicted end-to-end absmax rel err ~1.5e-4.

kernel(**inputs) takes full-size numpy inputs, returns [4,1024,512] float32.
"""
import sys
import math

sys.path.insert(0, "/opt/trn_rl_repo")

import numpy as np

import concourse.bass as bass
import concourse.bacc as bacc
import concourse.tile as tile
from concourse import bass_utils, mybir

dt = mybir.dt
AF = mybir.ActivationFunctionType
ALU = mybir.AluOpType

B, T, L, D = 4, 16, 1024, 512
NC = 8
LC = L // NC            # 128 L rows per core
ROWS = B * LC           # 512 fm rows per core (row = b*128 + l)
NCH = D // 128          # 4 feature-partition chunks
LAMBDA = 0.3
C_CONT = 1.0 / math.sqrt(D)
C_DRIFT = -LAMBDA / D
INV_L = 1.0 / L

_CACHE = {}


def _round_f32r(x):
    """Round fp32 to the FP32R grid (12-bit significand, RNE)."""
    x = np.ascontiguousarray(x, dtype=np.float32)
    b = x.view(np.uint32)
    lsb = (b >> np.uint32(12)) & np.uint32(1)
    out = (b + np.uint32(0x7FF) + lsb) & np.uint32(0xFFFFF000)
    return out.view(np.float32)


def _wdev(w):
    """[512,512] weight -> [128,2048] device layout (k-chunk c at cols c*512)."""
    return np.ascontiguousarray(
        w.reshape(4, 128, 512).transpose(1, 0, 2).reshape(128, 2048))


def _bias_fm(b):
    return np.ascontiguousarray(b.reshape(4, 128).T)


def _sin_table():
    pos = np.arange(1, T + 1, dtype=np.float32)
    half = D // 2
    div = np.exp(-math.log(10000.0) * (2.0 * np.arange(half, dtype=np.float32) / D))
    ang = pos[:, None] * div
    pe = np.stack([np.sin(ang), np.cos(ang)], axis=-1).reshape(T, D)
    return pe.astype(np.float32)


def _build(sim_mode=False, reps=1):
    nc = bacc.Bacc("TRN2", target_bir_lowering=False, debug=False,
                   num_devices=1 if sim_mode else NC)
    f32, f32r = dt.float32, dt.float32r

    MEM = nc.dram_tensor("MEM", [B, T, LC, D], f32r, kind="ExternalInput").ap()
    XPH = nc.dram_tensor("XPH", [B, LC, D], f32r, kind="ExternalInput").ap()
    XK = nc.dram_tensor("XK", [B, LC, D], f32r, kind="ExternalInput").ap()
    WR_names = ["wd", "wx", "wpn", "gx", "gp", "wo", "f1", "f2", "seqw"]
    WR = {n: nc.dram_tensor("W_" + n, [128, 2048], f32r, kind="ExternalInput").ap()
          for n in WR_names}
    WF_names = ["wm", "wmd", "wq", "wcd"]
    WF = {n: nc.dram_tensor("W_" + n, [128, 2048], f32, kind="ExternalInput").ap()
          for n in WF_names}
    BIAS = nc.dram_tensor("BIAS", [128, 36], f32, kind="ExternalInput").ap()
    SEQB = nc.dram_tensor("SEQB", [1, 512], f32r, kind="ExternalInput").ap()
    SINT = nc.dram_tensor("SINT", [128, 64], f32r, kind="ExternalInput").ap()
    IDENT = nc.dram_tensor("IDENT", [128, 128], f32, kind="ExternalInput").ap()
    IDENTR = nc.dram_tensor("IDENTR", [128, 128], f32r, kind="ExternalInput").ap()
    ONESC = nc.dram_tensor("ONESC", [128, 1], f32r, kind="ExternalInput").ap()
    ONESR = nc.dram_tensor("ONESR", [1, 128], f32r, kind="ExternalInput").ap()
    IDENTN = nc.dram_tensor("IDENTN", [128, 128], f32r, kind="ExternalInput").ap()
    OUT = nc.dram_tensor("OUT", [B, LC, D], f32, kind="ExternalOutput").ap()

    BI = {n: i for i, n in enumerate(
        ["b_A", "b_t1", "gate_b", "outp_b", "q_b", "mem_b", "curd_b",
         "memd_b", "fuse_b"])}

    def _emit(tc):
        with tc.tile_pool(name="sb", bufs=1) as sb, \
             tc.tile_pool(name="ps", bufs=1, space="PSUM") as ps, \
             tc.tile_pool(name="dram", bufs=1, space="DRAM") as dram:

            def S(shape, dtype, tag, bufs=1):
                return sb.tile(shape, dtype, tag=tag, bufs=bufs, name=tag)

            def P(shape, tag, bufs=1, dtype=dt.float32):
                return ps.tile(shape, dtype, tag=tag, bufs=bufs, name=tag)

            # ---------------- constants ----------------
            ident = S([128, 128], f32, "ident")
            identr = S([128, 128], f32r, "identr")
            onesc = S([128, 1], f32r, "onesc")
            onesr = S([1, 128], f32r, "onesr")
            biases = S([128, 36], f32, "biases")
            seqb = S([1, 512], f32r, "seqb")
            sint = S([128, 64], f32r, "sint")
            identn = S([128, 128], f32r, "identn")
            nc.sync.dma_start(ident, IDENT)
            nc.sync.dma_start(identr, IDENTR)
            nc.sync.dma_start(onesc, ONESC)
            nc.sync.dma_start(onesr, ONESR)
            nc.sync.dma_start(biases, BIAS)
            nc.sync.dma_start(seqb, SEQB)
            nc.sync.dma_start(sint, SINT)
            nc.sync.dma_start(identn, IDENTN)

            def bias_col(name):
                return biases[:, BI[name]:BI[name] + 1]

            # ---------------- input loads ----------------
            x_nat, xp_nat = [], []
            for b in range(B):
                xt = S([128, 512], f32r, "xnat", bufs=1)
                nc.sync.dma_start(xt, XK[b])
                x_nat.append(xt)
                pt_ = S([128, 512], f32r, "xpnat", bufs=1)
                nc.sync.dma_start(pt_, XPH[b])
                xp_nat.append(pt_)

            mem_res = {}

            def load_mem(b, part):
                if b < 2:
                    mt = S([128, 8 * 512], f32r, f"m{b}h{part}")
                    nc.sync.dma_start(
                        mt,
                        MEM[b, 8 * part:8 * (part + 1)].rearrange("t l d -> l t d"))
                else:
                    mt = S([128, 4 * 512], f32r, "ms", bufs=4)
                    nc.sync.dma_start(
                        mt,
                        MEM[b, 4 * part:4 * (part + 1)].rearrange("t l d -> l t d"))
                mem_res[(b, part)] = mt

            for b in range(2):
                for h in range(2):
                    load_mem(b, h)

            def mem_slice(b, t):
                if b < 2:
                    return mem_res[(b, t // 8)][:, (t % 8) * 512:(t % 8 + 1) * 512]
                return mem_res[(b, t // 4)][:, (t % 4) * 512:(t % 4 + 1) * 512]

            # ---------------- fm transposes ----------------
            x_fm = [S([128, ROWS], f32r, "xfm", bufs=4) for _ in range(NCH)]
            for b in range(B):
                for c in range(NCH):
                    pt = P([128, 128], "pt", bufs=1, dtype=f32r)
                    nc.tensor.transpose(pt, x_nat[b][:, c * 128:(c + 1) * 128],
                                        identr)
                    nc.scalar.copy(x_fm[c][:, b * 128:(b + 1) * 128], pt)
            xp_fm = [S([128, ROWS], f32r, "quadA", bufs=4) for _ in range(NCH)]
            for b in range(B):
                for c in range(NCH):
                    pt = P([128, 128], "pt", bufs=1, dtype=f32r)
                    nc.tensor.transpose(pt, xp_nat[b][:, c * 128:(c + 1) * 128],
                                        identr)
                    nc.vector.tensor_copy(xp_fm[c][:, b * 128:(b + 1) * 128], pt)

            ar_in = dram.tile([72, 512], f32, tag="ar_in", name="ar_in")
            ar_out = dram.tile([72, 512], f32, tag="ar_out", name="ar_out")

            # ---------------- L-sum colsums ----------------
            def emit_lsum(b, t):
                psum = P([1, 512], "pcs", bufs=2)
                nc.tensor.matmul(psum, onesc, mem_slice(b, t), start=True,
                                 stop=True)
                st = S([1, 512], f32, "stage", bufs=3)
                nc.vector.tensor_copy(st, psum)
                bt = b * T + t
                nc.scalar.dma_start(ar_in[bt:bt + 1, :], st)

            for t in range(8):
                emit_lsum(0, t)

            # ---------------- weight groups ----------------
            def load_w(name, table=WR, dtype=f32r):
                ta = sb.tile([128, 1024], dtype, tag="w", bufs=4, name="wa_" + name)
                tb = sb.tile([128, 1024], dtype, tag="w", bufs=4, name="wb_" + name)
                nc.scalar.dma_start(ta, table[name][:, 0:1024])
                nc.scalar.dma_start(tb, table[name][:, 1024:2048])
                return (ta, tb)

            def w_chunk(wt, c_k, c_out):
                half = wt[c_k // 2]
                off = (c_k % 2) * 512 + c_out * 128
                return half[:, off:off + 128]

            def mm_group(pairs, out_maker, n=ROWS):
                for c_out in range(NCH):
                    psum = P([128, n], "pmm", bufs=4)
                    first = True
                    for pi, (wt, rhs_list) in enumerate(pairs):
                        for c_k in range(NCH):
                            last = (pi == len(pairs) - 1) and (c_k == NCH - 1)
                            nc.tensor.matmul(
                                psum, w_chunk(wt, c_k, c_out), rhs_list[c_k],
                                start=first, stop=last)
                            first = False
                    out_maker(c_out, psum)

            # delta
            delta_fm = []
            for c in range(NCH):
                dfm = S([128, ROWS], f32r, "quadB", bufs=4)
                nc.vector.tensor_tensor(out=dfm, in0=x_fm[c], in1=xp_fm[c],
                                        op=ALU.subtract)
                delta_fm.append(dfm)

            dsum = [S([128, 4], f32, f"dsum{c}") for c in range(NCH)]
            for c in range(NCH):
                for b in range(B):
                    nc.vector.reduce_sum(
                        out=dsum[c][:, b:b + 1],
                        in_=delta_fm[c][:, b * 128:(b + 1) * 128],
                        axis=mybir.AxisListType.X)

            wx = load_w("wx")
            wpn = load_w("wpn")
            t1 = [None] * NCH

            def mk_t1(c, psum):
                o = S([128, ROWS], f32r, "feat", bufs=10)
                nc.scalar.activation(o, psum, AF.Identity, bias=bias_col("b_t1"))
                t1[c] = o
            mm_group([(wx, x_fm), (wpn, xp_fm)], mk_t1)

            for t in range(8, 16):
                emit_lsum(0, t)

            # A = delta@Wd - t1 + (delta_b - b_t1); then mid folds in place
            wd = load_w("wd")
            afeat = [None] * NCH
            for c_out in range(NCH):
                psum = P([128, ROWS], "pmm", bufs=4)
                for c_k in range(NCH):
                    nc.tensor.matmul(psum, w_chunk(wd, c_k, c_out),
                                     delta_fm[c_k], start=(c_k == 0), stop=False)
                nc.tensor.matmul(psum, identn, t1[c_out], start=False, stop=True)
                o = S([128, ROWS], f32r, "feat", bufs=10)
                nc.scalar.activation(o, psum, AF.Identity, bias=bias_col("b_A"))
                afeat[c_out] = o

            for t in range(8):
                emit_lsum(1, t)

            # phase A for b=3 memory: L-sums only (streamed via single slot)
            load_mem(3, 0)

            gxw = load_w("gx")
            gpw = load_w("gp")
            gsig = [None] * NCH

            def mk_g(c, psum):
                o = S([128, ROWS], f32r, "feat", bufs=10)
                nc.scalar.activation(o, psum, AF.Sigmoid, bias=bias_col("gate_b"))
                gsig[c] = o
            mm_group([(gxw, x_fm), (gpw, xp_fm)], mk_g)

            for t in range(8, 16):
                emit_lsum(1, t)
            load_mem(2, 0)
            for t in range(4):
                emit_lsum(2, t)
            load_mem(2, 1)
            for t in range(4, 8):
                emit_lsum(2, t)

            # mid = t1 + g*A  (in place in afeat)
            mid = afeat
            for c in range(NCH):
                nc.vector.tensor_tensor(out=afeat[c], in0=afeat[c], in1=gsig[c],
                                        op=ALU.mult)
                nc.vector.tensor_tensor(out=afeat[c], in0=afeat[c], in1=t1[c],
                                        op=ALU.add)

            load_mem(2, 2)
            for t in range(8, 12):
                emit_lsum(2, t)
            load_mem(2, 3)
            for t in range(12, 16):
                emit_lsum(2, t)
            for t in range(4):
                emit_lsum(3, t)
            load_mem(3, 1)
            for t in range(4, 8):
                emit_lsum(3, t)
            load_mem(3, 2)
            for t in range(8, 12):
                emit_lsum(3, t)
            load_mem(3, 3)
            for t in range(12, 16):
                emit_lsum(3, t)

            wo = load_w("wo")
            raw_fm = [None] * NCH

            def mk_raw(c, psum):
                o = S([128, ROWS], f32, "raw", bufs=4)
                nc.scalar.activation(o, psum, AF.Identity, bias=bias_col("outp_b"))
                raw_fm[c] = o
            mm_group([(wo, mid)], mk_raw)

            qsum = [S([128, 4], f32, f"qsum{c}") for c in range(NCH)]
            for c in range(NCH):
                xr = S([128, ROWS], f32, "feat", bufs=10)
                nc.vector.tensor_tensor(out=xr, in0=x_fm[c], in1=raw_fm[c],
                                        op=ALU.add)
                for b in range(B):
                    nc.vector.reduce_sum(
                        out=qsum[c][:, b:b + 1],
                        in_=xr[:, b * 128:(b + 1) * 128],
                        axis=mybir.AxisListType.X)

            # pos_emb natural [16,512] (f32r) — AR-independent, compute early
            seqw = load_w("seqw")
            pe_psum = P([16, 512], "pmm", bufs=4)
            for c_k in range(NCH):
                nc.tensor.matmul(pe_psum, sint[:, c_k * 16:(c_k + 1) * 16],
                                 seqw[c_k // 2][:, (c_k % 2) * 512:
                                                (c_k % 2) * 512 + 512],
                                 start=(c_k == 0), stop=False)
            nc.tensor.matmul(pe_psum, onesr[:, 0:16], seqb, start=False, stop=True)
            pe_nat = S([16, 512], f32r, "pe_nat")
            nc.vector.tensor_copy(pe_nat, pe_psum)

            qn = S([4, 512], f32, "qn")
            dn = S([4, 512], f32, "dn")
            for c in range(NCH):
                pt = P([128, 128], "pt", bufs=1)
                nc.tensor.transpose(pt[0:4, :], qsum[c], ident)
                nc.vector.tensor_copy(qn[:, c * 128:(c + 1) * 128], pt[0:4, :])
                pt2 = P([128, 128], "pt", bufs=1)
                nc.tensor.transpose(pt2[0:4, :], dsum[c], ident)
                nc.vector.tensor_copy(dn[:, c * 128:(c + 1) * 128], pt2[0:4, :])
            nc.scalar.dma_start(ar_in[64:68, :], qn)
            nc.scalar.dma_start(ar_in[68:72, :], dn)

            # ---------------- AllReduce ----------------
            if sim_mode:
                nc.sync.dma_start(ar_out, ar_in)
            else:
                nc.gpsimd.collective_compute(
                    "AllReduce", ALU.add,
                    replica_groups=[list(range(NC))],
                    ins=[ar_in[:]], outs=[ar_out[:]])

            f1w = load_w("f1")
            S_m = S([64, 512], f32, "S_m")
            qrows = S([4, 512], f32, "qrows")
            drows = S([4, 512], f32, "drows")
            nc.scalar.dma_start(S_m, ar_out[0:64, :])
            nc.scalar.dma_start(qrows, ar_out[64:68, :])
            nc.scalar.dma_start(drows, ar_out[68:72, :])

            # mean_fm + pos broadcast; md_fm diffs
            mean_fm, md_fm = [], []
            for c in range(NCH):
                pt = P([128, 128], "pt", bufs=1)
                nc.tensor.transpose(pt[:, 0:64], S_m[:, c * 128:(c + 1) * 128],
                                    ident[0:64, 0:64])
                mf = S([128, 64], f32, f"meanfm{c}")
                nc.scalar.activation(mf, pt[:, 0:64], AF.Copy, scale=INV_L)
                pt2 = P([128, 128], "pt", bufs=1)
                nc.tensor.transpose(pt2[:, 0:16],
                                    pe_nat.bitcast(f32)[:, c * 128:(c + 1) * 128],
                                    ident[0:16, 0:16])
                pf = S([128, 16], f32, f"posfm{c}")
                nc.vector.tensor_copy(pf, pt2[:, 0:16])
                bc = bass.AP(tensor=pf.tensor, offset=pf.offset,
                             ap=[pf.ap[0], [0, 4], [1, 16]])
                nc.vector.tensor_tensor(
                    out=mf.rearrange("p (b t) -> p b t", b=4),
                    in0=mf.rearrange("p (b t) -> p b t", b=4),
                    in1=bc, op=ALU.add)
                mean_fm.append(mf)
                md = S([128, 64], f32, f"mdfm{c}")
                for b in range(B):
                    o = b * 16
                    nc.vector.tensor_copy(md[:, o:o + 1], mf[:, o:o + 1])
                    nc.vector.tensor_tensor(
                        out=md[:, o + 1:o + 16],
                        in0=mf[:, o + 1:o + 16],
                        in1=mf[:, o:o + 15], op=ALU.subtract)
                md_fm.append(md)

            def small_group(wt, rhs_list, bias_name, n):
                outs = []
                for c_out in range(NCH):
                    psum = P([128, n], "pmm", bufs=4)
                    for c_k in range(NCH):
                        nc.tensor.matmul(
                            psum, w_chunk(wt, c_k, c_out), rhs_list[c_k],
                            start=(c_k == 0), stop=(c_k == NCH - 1))
                    o = S([128, n], f32, f"sg_{bias_name}{c_out}")
                    nc.scalar.activation(o, psum, AF.Identity, bias=bias_col(bias_name))
                    outs.append(o)
                return outs

            wm = load_w("wm", WF, f32)
            gm = small_group(wm, mean_fm, "mem_b", 64)
            wmd = load_w("wmd", WF, f32)
            dm = small_group(wmd, md_fm, "memd_b", 64)

            qin, cin = [], []
            for c in range(NCH):
                pt = P([128, 128], "pt", bufs=1)
                nc.tensor.transpose(pt[:, 0:4], qrows[:, c * 128:(c + 1) * 128],
                                    ident[0:4, 0:4])
                qi = S([128, 4], f32, f"qin{c}")
                nc.scalar.activation(qi, pt[:, 0:4], AF.Copy, scale=INV_L)
                qin.append(qi)
                pt2 = P([128, 128], "pt", bufs=1)
                nc.tensor.transpose(pt2[:, 0:4], drows[:, c * 128:(c + 1) * 128],
                                    ident[0:4, 0:4])
                ci = S([128, 4], f32, f"cin{c}")
                nc.scalar.activation(ci, pt2[:, 0:4], AF.Copy, scale=INV_L)
                cin.append(ci)
            wq = load_w("wq", WF, f32)
            qg = small_group(wq, qin, "q_b", 4)
            wcd = load_w("wcd", WF, f32)
            cd = small_group(wcd, cin, "curd_b", 4)

            # scores
            cont_ps = P([1, 64], "pcs", bufs=2)
            for c in range(NCH):
                pr = S([128, 64], f32r, "sc64", bufs=2)
                qb = bass.AP(tensor=qg[c].tensor, offset=qg[c].offset,
                             ap=[qg[c].ap[0], [1, 4], [0, 16]])
                nc.vector.tensor_tensor(
                    out=pr.rearrange("p (b t) -> p b t", b=4),
                    in0=gm[c].rearrange("p (b t) -> p b t", b=4),
                    in1=qb, op=ALU.mult)
                nc.tensor.matmul(cont_ps, onesc, pr, start=(c == 0),
                                 stop=(c == NCH - 1))
            sq_ps = P([1, 64], "pcs", bufs=2)
            for c in range(NCH):
                dd = S([128, 64], f32, "sc64", bufs=2)
                cb = bass.AP(tensor=cd[c].tensor, offset=cd[c].offset,
                             ap=[cd[c].ap[0], [1, 4], [0, 16]])
                nc.vector.tensor_tensor(
                    out=dd.rearrange("p (b t) -> p b t", b=4),
                    in0=dm[c].rearrange("p (b t) -> p b t", b=4),
                    in1=cb, op=ALU.subtract)
                sq = S([128, 64], f32r, "sc64", bufs=2)
                nc.vector.tensor_tensor(out=sq, in0=dd, in1=dd, op=ALU.mult)
                nc.tensor.matmul(sq_ps, onesc, sq, start=(c == 0),
                                 stop=(c == NCH - 1))

            score = S([1, 64], f32, "score")
            tmp_s = S([1, 64], f32, "tmp_s")
            nc.vector.tensor_scalar_mul(score, cont_ps, C_CONT)
            nc.vector.tensor_scalar_mul(tmp_s, sq_ps, C_DRIFT)
            nc.vector.tensor_tensor(out=score, in0=score, in1=tmp_s, op=ALU.add)
            mx = S([1, 4], f32, "mx")
            nc.vector.reduce_max(out=mx,
                                 in_=score.rearrange("p (b t) -> p b t", b=4),
                                 axis=mybir.AxisListType.X)
            mxb = bass.AP(tensor=mx.tensor, offset=mx.offset,
                          ap=[mx.ap[0], [1, 4], [0, 16]])
            sc2 = S([1, 64], f32, "sc2")
            nc.vector.tensor_tensor(out=sc2.rearrange("p (b t) -> p b t", b=4),
                                    in0=score.rearrange("p (b t) -> p b t", b=4),
                                    in1=mxb, op=ALU.subtract)
            ex = S([1, 64], f32, "ex")
            nc.scalar.activation(ex, sc2, AF.Exp)
            sm = S([1, 4], f32, "sm")
            nc.vector.reduce_sum(out=sm, in_=ex.rearrange("p (b t) -> p b t", b=4),
                                 axis=mybir.AxisListType.X)
            rs = S([1, 4], f32, "rs")
            nc.vector.reciprocal(rs, sm)
            rsb = bass.AP(tensor=rs.tensor, offset=rs.offset,
                          ap=[rs.ap[0], [1, 4], [0, 16]])
            attn = S([1, 64], f32r, "attn")
            nc.vector.tensor_tensor(out=attn.rearrange("p (b t) -> p b t", b=4),
                                    in0=ex.rearrange("p (b t) -> p b t", b=4),
                                    in1=rsb, op=ALU.mult)

            attn_dr = dram.tile([1, 64], f32r, tag="attn_dr", name="attn_dr")
            nc.scalar.dma_start(attn_dr, attn)
            attn_t4 = S([16, 4], f32r, "attn_t4")
            rd = bass.AP(tensor=attn_dr.tensor, offset=attn_dr.offset,
                         ap=[[1, 16], [16, 4]])
            nc.scalar.dma_start(attn_t4, rd)

            ab_ps = P([128, 64], "pmm", bufs=4)
            nc.tensor.matmul(ab_ps, onesr, attn, start=True, stop=True)
            ab = S([128, 64], f32, "ab")
            nc.vector.tensor_copy(ab, ab_ps)

            # ---------------- enhanced ----------------
            enh_nat = []
            for b in range(B):
                if b >= 2:
                    load_mem(b, 0)
                    load_mem(b, 1)
                ep = P([128, 512], "pmm", bufs=4)
                for t in range(T):
                    if b >= 2 and t in (4, 8) :
                        load_mem(b, t // 4 + 1)
                    dg = S([128, 128], f32r, "diag", bufs=2)
                    nc.vector.tensor_scalar_mul(dg, ident,
                                                ab[:, b * T + t:b * T + t + 1])
                    nc.tensor.matmul(ep, dg, mem_slice(b, t),
                                     start=(t == 0), stop=False)
                pc_ps = P([1, 512], "pcs", bufs=2)
                nc.tensor.matmul(pc_ps, attn_t4[:, b:b + 1], pe_nat,
                                 start=True, stop=True)
                pc_sb = S([1, 512], f32r, "stage2", bufs=1)
                nc.vector.tensor_copy(pc_sb, pc_ps)
                nc.tensor.matmul(ep, onesr, pc_sb, start=False, stop=True)
                en = S([128, 512], f32, "quadA", bufs=4)
                nc.vector.tensor_copy(en, ep)
                enh_nat.append(en)

            enh_fm = [S([128, ROWS], f32r, "quadB", bufs=4) for _ in range(NCH)]
            for b in range(B):
                for c in range(NCH):
                    pt = P([128, 128], "pt", bufs=1)
                    nc.tensor.transpose(pt, enh_nat[b][:, c * 128:(c + 1) * 128],
                                        ident)
                    nc.scalar.copy(enh_fm[c][:, b * 128:(b + 1) * 128], pt)

            # fuse + output (logits = x@F1 + enh@F2 in one psum group)
            f2w = load_w("f2")
            for c_out in range(NCH):
                psum = P([128, ROWS], "pmm", bufs=4)
                for c_k in range(NCH):
                    nc.tensor.matmul(
                        psum, w_chunk(f1w, c_k, c_out), x_fm[c_k],
                        start=(c_k == 0), stop=False)
                for c_k in range(NCH):
                    nc.tensor.matmul(
                        psum, w_chunk(f2w, c_k, c_out), enh_fm[c_k],
                        start=False, stop=(c_k == NCH - 1))
                fg = S([128, ROWS], f32, "feat", bufs=10)
                nc.scalar.activation(fg, psum, AF.Sigmoid, bias=bias_col("fuse_b"))
                prod = S([128, ROWS], f32, "feat", bufs=10)
                nc.vector.tensor_tensor(out=prod, in0=fg, in1=enh_fm[c_out],
                                        op=ALU.mult)
                s1 = S([128, ROWS], f32, "feat", bufs=10)
                nc.vector.tensor_tensor(out=s1, in0=prod, in1=raw_fm[c_out],
                                        op=ALU.add)
                ofm = S([128, ROWS], f32, "feat", bufs=10)
                nc.vector.tensor_tensor(out=ofm, in0=s1, in1=x_fm[c_out],
                                        op=ALU.add)
                for b in range(B):
                    pt = P([128, 128], "pt", bufs=1)
                    nc.tensor.transpose(pt, ofm[:, b * 128:(b + 1) * 128], ident)
                    on = S([128, 128], f32, "onat", bufs=2)
                    nc.vector.tensor_copy(on, pt)
                    nc.sync.dma_start(OUT[b][:, c_out * 128:(c_out + 1) * 128], on)

    with tile.TileContext(nc) as tc:
        for _ in range(reps):
            _emit(tc)

    nc.compile()
    return nc


def _prep_maps(inputs):
    x = np.ascontiguousarray(inputs["x"], dtype=np.float32)
    mem = np.ascontiguousarray(inputs["memory_snapshot"], dtype=np.float32)

    gw = np.asarray(inputs["gate_W"], np.float32)
    fw = np.asarray(inputs["fuse_W"], np.float32)
    r = _round_f32r
    weights_r = {
        "wd": r(np.asarray(inputs["delta_W"], np.float32)),
        "wx": r(np.asarray(inputs["xproj_W"], np.float32)),
        "wpn": r(-np.asarray(inputs["phys_W"], np.float32)),
        "gx": r(gw[0:512] + gw[512:1024]),
        "gp": r(gw[1024:1536] - gw[0:512]),
        "wo": r(np.asarray(inputs["outp_W"], np.float32)),
        "f1": r(fw[0:512]),
        "f2": r(fw[512:1024]),
        "seqw": r(np.asarray(inputs["seq_W"], np.float32)),
    }
    weights_f = {
        "wm": np.asarray(inputs["mem_W"], np.float32),
        "wmd": np.asarray(inputs["memd_W"], np.float32),
        "wq": np.asarray(inputs["q_W"], np.float32),
        "wcd": np.asarray(inputs["curd_W"], np.float32),
    }
    b_t1_v = (np.asarray(inputs["xproj_b"], np.float32)
              - np.asarray(inputs["phys_b"], np.float32))
    bias_mat = np.stack([
        _bias_fm(np.asarray(inputs["delta_b"], np.float32) - b_t1_v),
        _bias_fm(b_t1_v),
        _bias_fm(np.asarray(inputs["gate_b"], np.float32)),
        _bias_fm(np.asarray(inputs["outp_b"], np.float32)),
        _bias_fm(np.asarray(inputs["q_b"], np.float32)),
        _bias_fm(np.asarray(inputs["mem_b"], np.float32)),
        _bias_fm(np.asarray(inputs["curd_b"], np.float32)),
        _bias_fm(np.asarray(inputs["memd_b"], np.float32)),
        _bias_fm(np.asarray(inputs["fuse_b"], np.float32)),
    ], axis=1).reshape(128, 36)

    sin_t = _sin_table()
    sint_dev = np.zeros((128, 64), np.float32)
    for c in range(4):
        sint_dev[:, c * 16:(c + 1) * 16] = sin_t[:, c * 128:(c + 1) * 128].T

    shared = {("W_" + n): _wdev(w) for n, w in weights_r.items()}
    shared.update({("W_" + n): _wdev(w) for n, w in weights_f.items()})
    shared.update({
        "BIAS": np.ascontiguousarray(bias_mat),
        "SEQB": r(np.asarray(inputs["seq_b"], np.float32)).reshape(1, 512),
        "SINT": r(sint_dev),
        "IDENT": np.eye(128, dtype=np.float32),
        "IDENTR": np.eye(128, dtype=np.float32),
        "ONESC": np.ones((128, 1), np.float32),
        "ONESR": np.ones((1, 128), np.float32),
        "IDENTN": -np.eye(128, dtype=np.float32),
    })

    mem_r = _round_f32r(mem)
    x_r = _round_f32r(x)
    in_maps = []
    for k in range(NC):
        sl = slice(k * LC, (k + 1) * LC)
        m = dict(shared)
        m["MEM"] = np.ascontiguousarray(mem_r[:, :, sl, :])
        m["XPH"] = np.ascontiguousarray(mem_r[:, T - 1, sl, :])
        m["XK"] = np.ascontiguousarray(x_r[:, sl, :])
        in_maps.append(m)
    return in_maps


def kernel(**inputs):
    if "nc" not in _CACHE:
        _CACHE["nc"] = _build()
    ncb = _CACHE["nc"]
    in_maps = _prep_maps(inputs)
    res = bass_utils.run_bass_kernel_spmd(ncb, in_maps, core_ids=list(range(NC)))
    out = np.empty((B, L, D), np.float32)
    for k in range(NC):
        out[:, k * LC:(k + 1) * LC, :] = res.results[k]["OUT"]
    return out


# revision 19
# speedup vs baseline: 734.8676x; 2.1702x over previous
"""DriftAwareLightMemory fused Bass/Tile kernel for 8 trn2 NeuronCores.

Strategy (L-sharded data parallel):
  - Shard the sequence axis L=1024 into 8 chunks of 128 rows; each core gets
    x[:, k*128:(k+1)*128] and memory_snapshot[:, :, k*128:(k+1)*128].
  - Each core keeps its 16 MB memory chunk resident in SBUF, computes
    per-(b,t) column sums over its L rows (for the means), the per-row
    DriftCorrectionExtractor matmuls, and partial sums for q_global/cur_drift.
  - One 147 KB 8-core AllReduce combines the partials; every core then
    redundantly computes the tiny time-attention softmax and finishes its
    L-chunk: enhanced = sum_t attn[b,t]*memory[b,t] via PSUM-accumulated diag
    matmuls over the SBUF-resident chunk, then the fuse gate and output.
  - Matmuls run as float32r (fp32 with 12-bit significand): full PE speed,
    ~1.2e-4 input rounding; predicted end-to-end absmax rel err ~1.5e-4.

kernel(**inputs) takes full-size numpy inputs, returns [4,1024,512] float32.
"""
import sys
import math

sys.path.insert(0, "/opt/trn_rl_repo")

import numpy as np

import concourse.bass as bass
import concourse.bacc as bacc
import concourse.tile as tile
from concourse import bass_utils, mybir

dt = mybir.dt
AF = mybir.ActivationFunctionType
ALU = mybir.AluOpType

B, T, L, D = 4, 16, 1024, 512
NC = 8
LC = L // NC            # 128 L rows per core
ROWS = B * LC           # 512 fm rows per core (row = b*128 + l)
NCH = D // 128          # 4 feature-partition chunks
LAMBDA = 0.3
C_CONT = 1.0 / math.sqrt(D)
C_DRIFT = -LAMBDA / D
INV_L = 1.0 / L

_CACHE = {}


def _round_f32r(x):
    """Round fp32 to the FP32R grid (12-bit significand, RNE)."""
    x = np.ascontiguousarray(x, dtype=np.float32)
    b = x.view(np.uint32)
    lsb = (b >> np.uint32(12)) & np.uint32(1)
    out = (b + np.uint32(0x7FF) + lsb) & np.uint32(0xFFFFF000)
    return out.view(np.float32)


def _wdev(w):
    """[512,512] weight -> [128,2048] device layout (k-chunk c at cols c*512)."""
    return np.ascontiguousarray(
        w.reshape(4, 128, 512).transpose(1, 0, 2).reshape(128, 2048))


def _bias_fm(b):
    return np.ascontiguousarray(b.reshape(4, 128).T)


def _sin_table():
    pos = np.arange(1, T + 1, dtype=np.float32)
    half = D // 2
    div = np.exp(-math.log(10000.0) * (2.0 * np.arange(half, dtype=np.float32) / D))
    ang = pos[:, None] * div
    pe = np.stack([np.sin(ang), np.cos(ang)], axis=-1).reshape(T, D)
    return pe.astype(np.float32)


def _build(sim_mode=False, reps=1, fake_ar=None):
    if fake_ar is None:
        fake_ar = sim_mode
    nc = bacc.Bacc("TRN2", target_bir_lowering=False, debug=False,
                   num_devices=1 if sim_mode else NC)
    f32, f32r = dt.float32, dt.float32r

    MEM = nc.dram_tensor("MEM", [B, T, LC, D], f32r, kind="ExternalInput").ap()
    XPH = nc.dram_tensor("XPH", [B, LC, D], f32r, kind="ExternalInput").ap()
    XK = nc.dram_tensor("XK", [B, LC, D], f32r, kind="ExternalInput").ap()
    WR_names = ["wd", "wx", "wpn", "gx", "gp", "wo", "f1", "f2", "seqw"]
    WR = {n: nc.dram_tensor("W_" + n, [128, 2048], f32r, kind="ExternalInput").ap()
          for n in WR_names}
    WF_names = ["wm", "wmd", "wq", "wcd"]
    WF = {n: nc.dram_tensor("W_" + n, [128, 2048], f32, kind="ExternalInput").ap()
          for n in WF_names}
    BIAS = nc.dram_tensor("BIAS", [128, 36], f32, kind="ExternalInput").ap()
    SEQB = nc.dram_tensor("SEQB", [1, 512], f32r, kind="ExternalInput").ap()
    SINT = nc.dram_tensor("SINT", [128, 64], f32r, kind="ExternalInput").ap()
    IDENT = nc.dram_tensor("IDENT", [128, 128], f32, kind="ExternalInput").ap()
    IDENTR = nc.dram_tensor("IDENTR", [128, 128], f32r, kind="ExternalInput").ap()
    ONESC = nc.dram_tensor("ONESC", [128, 1], f32r, kind="ExternalInput").ap()
    ONESR = nc.dram_tensor("ONESR", [1, 128], f32r, kind="ExternalInput").ap()
    IDENTN = nc.dram_tensor("IDENTN", [128, 128], f32r, kind="ExternalInput").ap()
    OUT = nc.dram_tensor("OUT", [B, LC, D], f32, kind="ExternalOutput").ap()

    BI = {n: i for i, n in enumerate(
        ["b_A", "b_t1", "gate_b", "outp_b", "q_b", "mem_b", "curd_b",
         "memd_b", "fuse_b"])}

    def _emit(tc):
        with tc.tile_pool(name="sb", bufs=1) as sb, \
             tc.tile_pool(name="ps", bufs=1, space="PSUM") as ps, \
             tc.tile_pool(name="dram", bufs=1, space="DRAM") as dram:

            def S(shape, dtype, tag, bufs=1):
                return sb.tile(shape, dtype, tag=tag, bufs=bufs, name=tag)

            def P(shape, tag, bufs=1, dtype=dt.float32):
                return ps.tile(shape, dtype, tag=tag, bufs=bufs, name=tag)

            # ---------------- constants ----------------
            ident = S([128, 128], f32, "ident")
            identr = S([128, 128], f32r, "identr")
            onesc = S([128, 1], f32r, "onesc")
            onesr = S([1, 128], f32r, "onesr")
            biases = S([128, 36], f32, "biases")
            seqb = S([1, 512], f32r, "seqb")
            sint = S([128, 64], f32r, "sint")
            identn = S([128, 128], f32r, "identn")
            nc.sync.dma_start(ident, IDENT)
            nc.sync.dma_start(identr, IDENTR)
            nc.sync.dma_start(onesc, ONESC)
            nc.sync.dma_start(onesr, ONESR)
            nc.sync.dma_start(biases, BIAS)
            nc.sync.dma_start(seqb, SEQB)
            nc.sync.dma_start(sint, SINT)
            nc.sync.dma_start(identn, IDENTN)

            def bias_col(name):
                return biases[:, BI[name]:BI[name] + 1]

            # ---------------- input loads ----------------
            x_nat, xp_nat = [], []
            for b in range(B):
                xt = S([128, 512], f32r, "xnat", bufs=1)
                nc.sync.dma_start(xt, XK[b])
                x_nat.append(xt)
                pt_ = S([128, 512], f32r, "xpnat", bufs=1)
                nc.sync.dma_start(pt_, XPH[b])
                xp_nat.append(pt_)

            mem_res = {}

            def load_mem(b, part):
                if b < 2:
                    mt = S([128, 8 * 512], f32r, f"m{b}h{part}")
                    nc.sync.dma_start(
                        mt,
                        MEM[b, 8 * part:8 * (part + 1)].rearrange("t l d -> l t d"))
                else:
                    mt = S([128, 4 * 512], f32r, "ms", bufs=4)
                    nc.sync.dma_start(
                        mt,
                        MEM[b, 4 * part:4 * (part + 1)].rearrange("t l d -> l t d"))
                mem_res[(b, part)] = mt

            for b in range(2):
                for h in range(2):
                    load_mem(b, h)

            def mem_slice(b, t):
                if b < 2:
                    return mem_res[(b, t // 8)][:, (t % 8) * 512:(t % 8 + 1) * 512]
                return mem_res[(b, t // 4)][:, (t % 4) * 512:(t % 4 + 1) * 512]

            # ---------------- fm transposes ----------------
            x_fm = [S([128, ROWS], f32r, "xfm", bufs=4) for _ in range(NCH)]
            for b in range(B):
                for c in range(NCH):
                    pt = P([128, 128], "pt", bufs=1, dtype=f32r)
                    nc.tensor.transpose(pt, x_nat[b][:, c * 128:(c + 1) * 128],
                                        identr)
                    nc.scalar.copy(x_fm[c][:, b * 128:(b + 1) * 128], pt)
            xp_fm = [S([128, ROWS], f32r, "quadA", bufs=4) for _ in range(NCH)]
            for b in range(B):
                for c in range(NCH):
                    pt = P([128, 128], "pt", bufs=1, dtype=f32r)
                    nc.tensor.transpose(pt, xp_nat[b][:, c * 128:(c + 1) * 128],
                                        identr)
                    nc.vector.tensor_copy(xp_fm[c][:, b * 128:(b + 1) * 128], pt)

            ar_in = dram.tile([72, 512], f32, tag="ar_in", name="ar_in")
            ar_out = dram.tile([72, 512], f32, tag="ar_out", name="ar_out")

            # ---------------- L-sum colsums ----------------
            def emit_lsum(b, t):
                psum = P([1, 512], "pcs", bufs=2)
                nc.tensor.matmul(psum, onesc, mem_slice(b, t), start=True,
                                 stop=True)
                st = S([1, 512], f32, "stage", bufs=3)
                nc.vector.tensor_copy(st, psum)
                bt = b * T + t
                nc.scalar.dma_start(ar_in[bt:bt + 1, :], st)

            for t in range(8):
                emit_lsum(0, t)

            # ---------------- weight groups ----------------
            def load_w(name, table=WR, dtype=f32r):
                ta = sb.tile([128, 1024], dtype, tag="w", bufs=4, name="wa_" + name)
                tb = sb.tile([128, 1024], dtype, tag="w", bufs=4, name="wb_" + name)
                nc.scalar.dma_start(ta, table[name][:, 0:1024])
                nc.scalar.dma_start(tb, table[name][:, 1024:2048])
                return (ta, tb)

            def w_chunk(wt, c_k, c_out):
                half = wt[c_k // 2]
                off = (c_k % 2) * 512 + c_out * 128
                return half[:, off:off + 128]

            def mm_group(pairs, out_maker, n=ROWS):
                for c_out in range(NCH):
                    psum = P([128, n], "pmm", bufs=4)
                    first = True
                    for pi, (wt, rhs_list) in enumerate(pairs):
                        for c_k in range(NCH):
                            last = (pi == len(pairs) - 1) and (c_k == NCH - 1)
                            nc.tensor.matmul(
                                psum, w_chunk(wt, c_k, c_out), rhs_list[c_k],
                                start=first, stop=last)
                            first = False
                    out_maker(c_out, psum)

            # delta
            delta_fm = []
            for c in range(NCH):
                dfm = S([128, ROWS], f32r, "quadB", bufs=4)
                nc.vector.tensor_tensor(out=dfm, in0=x_fm[c], in1=xp_fm[c],
                                        op=ALU.subtract)
                delta_fm.append(dfm)

            dsum = [S([128, 4], f32, f"dsum{c}") for c in range(NCH)]
            for c in range(NCH):
                for b in range(B):
                    nc.vector.reduce_sum(
                        out=dsum[c][:, b:b + 1],
                        in_=delta_fm[c][:, b * 128:(b + 1) * 128],
                        axis=mybir.AxisListType.X)

            wx = load_w("wx")
            wpn = load_w("wpn")
            t1 = [None] * NCH

            def mk_t1(c, psum):
                o = S([128, ROWS], f32r, "feat", bufs=10)
                nc.scalar.activation(o, psum, AF.Identity, bias=bias_col("b_t1"))
                t1[c] = o
            mm_group([(wx, x_fm), (wpn, xp_fm)], mk_t1)

            for t in range(8, 16):
                emit_lsum(0, t)

            # A = delta@Wd - t1 + (delta_b - b_t1); then mid folds in place
            wd = load_w("wd")
            afeat = [None] * NCH
            for c_out in range(NCH):
                psum = P([128, ROWS], "pmm", bufs=4)
                for c_k in range(NCH):
                    nc.tensor.matmul(psum, w_chunk(wd, c_k, c_out),
                                     delta_fm[c_k], start=(c_k == 0), stop=False)
                nc.tensor.matmul(psum, identn, t1[c_out], start=False, stop=True)
                o = S([128, ROWS], f32r, "feat", bufs=10)
                nc.scalar.activation(o, psum, AF.Identity, bias=bias_col("b_A"))
                afeat[c_out] = o

            for t in range(8):
                emit_lsum(1, t)

            # phase A for b=3 memory: L-sums only (streamed via single slot)
            load_mem(3, 0)

            gxw = load_w("gx")
            gpw = load_w("gp")
            gsig = [None] * NCH

            def mk_g(c, psum):
                o = S([128, ROWS], f32r, "feat", bufs=10)
                nc.scalar.activation(o, psum, AF.Sigmoid, bias=bias_col("gate_b"))
                gsig[c] = o
            mm_group([(gxw, x_fm), (gpw, xp_fm)], mk_g)

            for t in range(8, 16):
                emit_lsum(1, t)
            load_mem(2, 0)
            for t in range(4):
                emit_lsum(2, t)
            load_mem(2, 1)
            for t in range(4, 8):
                emit_lsum(2, t)

            # mid = t1 + g*A  (in place in afeat)
            mid = afeat
            for c in range(NCH):
                nc.vector.tensor_tensor(out=afeat[c], in0=afeat[c], in1=gsig[c],
                                        op=ALU.mult)
                nc.vector.tensor_tensor(out=afeat[c], in0=afeat[c], in1=t1[c],
                                        op=ALU.add)

            load_mem(2, 2)
            for t in range(8, 12):
                emit_lsum(2, t)
            load_mem(2, 3)
            for t in range(12, 16):
                emit_lsum(2, t)
            for t in range(4):
                emit_lsum(3, t)
            load_mem(3, 1)
            for t in range(4, 8):
                emit_lsum(3, t)
            load_mem(3, 2)
            for t in range(8, 12):
                emit_lsum(3, t)
            load_mem(3, 3)
            for t in range(12, 16):
                emit_lsum(3, t)

            wo = load_w("wo")
            raw_fm = [None] * NCH

            def mk_raw(c, psum):
                o = S([128, ROWS], f32, "raw", bufs=4)
                nc.scalar.activation(o, psum, AF.Identity, bias=bias_col("outp_b"))
                raw_fm[c] = o
            mm_group([(wo, mid)], mk_raw)

            qsum = [S([128, 4], f32, f"qsum{c}") for c in range(NCH)]
            for c in range(NCH):
                xr = S([128, ROWS], f32, "feat", bufs=10)
                nc.vector.tensor_tensor(out=xr, in0=x_fm[c], in1=raw_fm[c],
                                        op=ALU.add)
                for b in range(B):
                    nc.vector.reduce_sum(
                        out=qsum[c][:, b:b + 1],
                        in_=xr[:, b * 128:(b + 1) * 128],
                        axis=mybir.AxisListType.X)

            # pos_emb natural [16,512] (f32r) — AR-independent, compute early
            seqw = load_w("seqw")
            pe_psum = P([16, 512], "pmm", bufs=4)
            for c_k in range(NCH):
                nc.tensor.matmul(pe_psum, sint[:, c_k * 16:(c_k + 1) * 16],
                                 seqw[c_k // 2][:, (c_k % 2) * 512:
                                                (c_k % 2) * 512 + 512],
                                 start=(c_k == 0), stop=False)
            nc.tensor.matmul(pe_psum, onesr[:, 0:16], seqb, start=False, stop=True)
            pe_nat = S([16, 512], f32r, "pe_nat")
            nc.vector.tensor_copy(pe_nat, pe_psum)

            qn = S([4, 512], f32, "qn")
            dn = S([4, 512], f32, "dn")
            for c in range(NCH):
                pt = P([128, 128], "pt", bufs=1)
                nc.tensor.transpose(pt[0:4, :], qsum[c], ident)
                nc.vector.tensor_copy(qn[:, c * 128:(c + 1) * 128], pt[0:4, :])
                pt2 = P([128, 128], "pt", bufs=1)
                nc.tensor.transpose(pt2[0:4, :], dsum[c], ident)
                nc.vector.tensor_copy(dn[:, c * 128:(c + 1) * 128], pt2[0:4, :])
            nc.scalar.dma_start(ar_in[64:68, :], qn)
            nc.scalar.dma_start(ar_in[68:72, :], dn)

            # ---------------- AllReduce ----------------
            if fake_ar:
                nc.sync.dma_start(ar_out, ar_in)
            else:
                nc.gpsimd.collective_compute(
                    "AllReduce", ALU.add,
                    replica_groups=[list(range(NC))],
                    ins=[ar_in[:]], outs=[ar_out[:]])

            f1w = load_w("f1")
            f1log = [None] * NCH

            def mk_f1(c, psum):
                o = S([128, ROWS], f32, "feat", bufs=10)
                nc.vector.tensor_copy(o, psum)
                f1log[c] = o
            mm_group([(f1w, x_fm)], mk_f1)

            S_m = S([64, 512], f32, "S_m")
            qrows = S([4, 512], f32, "qrows")
            drows = S([4, 512], f32, "drows")
            nc.scalar.dma_start(S_m, ar_out[0:64, :])
            nc.scalar.dma_start(qrows, ar_out[64:68, :])
            nc.scalar.dma_start(drows, ar_out[68:72, :])

            # mean_fm + pos broadcast; md_fm diffs
            mean_fm, md_fm = [], []
            for c in range(NCH):
                pt = P([128, 128], "pt", bufs=1)
                nc.tensor.transpose(pt[:, 0:64], S_m[:, c * 128:(c + 1) * 128],
                                    ident[0:64, 0:64])
                mf = S([128, 64], f32, f"meanfm{c}")
                nc.scalar.activation(mf, pt[:, 0:64], AF.Copy, scale=INV_L)
                pt2 = P([128, 128], "pt", bufs=1)
                nc.tensor.transpose(pt2[:, 0:16],
                                    pe_nat.bitcast(f32)[:, c * 128:(c + 1) * 128],
                                    ident[0:16, 0:16])
                pf = S([128, 16], f32, f"posfm{c}")
                nc.vector.tensor_copy(pf, pt2[:, 0:16])
                bc = bass.AP(tensor=pf.tensor, offset=pf.offset,
                             ap=[pf.ap[0], [0, 4], [1, 16]])
                nc.vector.tensor_tensor(
                    out=mf.rearrange("p (b t) -> p b t", b=4),
                    in0=mf.rearrange("p (b t) -> p b t", b=4),
                    in1=bc, op=ALU.add)
                mean_fm.append(mf)
                md = S([128, 64], f32, f"mdfm{c}")
                for b in range(B):
                    o = b * 16
                    nc.vector.tensor_copy(md[:, o:o + 1], mf[:, o:o + 1])
                    nc.vector.tensor_tensor(
                        out=md[:, o + 1:o + 16],
                        in0=mf[:, o + 1:o + 16],
                        in1=mf[:, o:o + 15], op=ALU.subtract)
                md_fm.append(md)

            def small_group(wt, rhs_list, bias_name, n):
                outs = []
                for c_out in range(NCH):
                    psum = P([128, n], "pmm", bufs=4)
                    for c_k in range(NCH):
                        nc.tensor.matmul(
                            psum, w_chunk(wt, c_k, c_out), rhs_list[c_k],
                            start=(c_k == 0), stop=(c_k == NCH - 1))
                    o = S([128, n], f32, f"sg_{bias_name}{c_out}")
                    nc.scalar.activation(o, psum, AF.Identity, bias=bias_col(bias_name))
                    outs.append(o)
                return outs

            wm = load_w("wm", WF, f32)
            gm = small_group(wm, mean_fm, "mem_b", 64)
            wmd = load_w("wmd", WF, f32)
            dm = small_group(wmd, md_fm, "memd_b", 64)

            qin, cin = [], []
            for c in range(NCH):
                pt = P([128, 128], "pt", bufs=1)
                nc.tensor.transpose(pt[:, 0:4], qrows[:, c * 128:(c + 1) * 128],
                                    ident[0:4, 0:4])
                qi = S([128, 4], f32, f"qin{c}")
                nc.scalar.activation(qi, pt[:, 0:4], AF.Copy, scale=INV_L)
                qin.append(qi)
                pt2 = P([128, 128], "pt", bufs=1)
                nc.tensor.transpose(pt2[:, 0:4], drows[:, c * 128:(c + 1) * 128],
                                    ident[0:4, 0:4])
                ci = S([128, 4], f32, f"cin{c}")
                nc.scalar.activation(ci, pt2[:, 0:4], AF.Copy, scale=INV_L)
                cin.append(ci)
            wq = load_w("wq", WF, f32)
            qg = small_group(wq, qin, "q_b", 4)
            wcd = load_w("wcd", WF, f32)
            cd = small_group(wcd, cin, "curd_b", 4)

            # scores
            cont_ps = P([1, 64], "pcs", bufs=2)
            for c in range(NCH):
                pr = S([128, 64], f32r, "sc64", bufs=2)
                qb = bass.AP(tensor=qg[c].tensor, offset=qg[c].offset,
                             ap=[qg[c].ap[0], [1, 4], [0, 16]])
                nc.vector.tensor_tensor(
                    out=pr.rearrange("p (b t) -> p b t", b=4),
                    in0=gm[c].rearrange("p (b t) -> p b t", b=4),
                    in1=qb, op=ALU.mult)
                nc.tensor.matmul(cont_ps, onesc, pr, start=(c == 0),
                                 stop=(c == NCH - 1))
            sq_ps = P([1, 64], "pcs", bufs=2)
            for c in range(NCH):
                dd = S([128, 64], f32, "sc64", bufs=2)
                cb = bass.AP(tensor=cd[c].tensor, offset=cd[c].offset,
                             ap=[cd[c].ap[0], [1, 4], [0, 16]])
                nc.vector.tensor_tensor(
                    out=dd.rearrange("p (b t) -> p b t", b=4),
                    in0=dm[c].rearrange("p (b t) -> p b t", b=4),
                    in1=cb, op=ALU.subtract)
                sq = S([128, 64], f32r, "sc64", bufs=2)
                nc.vector.tensor_tensor(out=sq, in0=dd, in1=dd, op=ALU.mult)
                nc.tensor.matmul(sq_ps, onesc, sq, start=(c == 0),
                                 stop=(c == NCH - 1))

            score = S([1, 64], f32, "score")
            tmp_s = S([1, 64], f32, "tmp_s")
            nc.vector.tensor_scalar_mul(score, cont_ps, C_CONT)
            nc.vector.tensor_scalar_mul(tmp_s, sq_ps, C_DRIFT)
            nc.vector.tensor_tensor(out=score, in0=score, in1=tmp_s, op=ALU.add)
            mx = S([1, 4], f32, "mx")
            nc.vector.reduce_max(out=mx,
                                 in_=score.rearrange("p (b t) -> p b t", b=4),
                                 axis=mybir.AxisListType.X)
            mxb = bass.AP(tensor=mx.tensor, offset=mx.offset,
                          ap=[mx.ap[0], [1, 4], [0, 16]])
            sc2 = S([1, 64], f32, "sc2")
            nc.vector.tensor_tensor(out=sc2.rearrange("p (b t) -> p b t", b=4),
                                    in0=score.rearrange("p (b t) -> p b t", b=4),
                                    in1=mxb, op=ALU.subtract)
            ex = S([1, 64], f32, "ex")
            nc.scalar.activation(ex, sc2, AF.Exp)
            sm = S([1, 4], f32, "sm")
            nc.vector.reduce_sum(out=sm, in_=ex.rearrange("p (b t) -> p b t", b=4),
                                 axis=mybir.AxisListType.X)
            rs = S([1, 4], f32, "rs")
            nc.vector.reciprocal(rs, sm)
            rsb = bass.AP(tensor=rs.tensor, offset=rs.offset,
                          ap=[rs.ap[0], [1, 4], [0, 16]])
            attn = S([1, 64], f32r, "attn")
            nc.vector.tensor_tensor(out=attn.rearrange("p (b t) -> p b t", b=4),
                                    in0=ex.rearrange("p (b t) -> p b t", b=4),
                                    in1=rsb, op=ALU.mult)

            attn_dr = dram.tile([1, 64], f32r, tag="attn_dr", name="attn_dr")
            nc.scalar.dma_start(attn_dr, attn)
            attn_t4 = S([16, 4], f32r, "attn_t4")
            rd = bass.AP(tensor=attn_dr.tensor, offset=attn_dr.offset,
                         ap=[[1, 16], [16, 4]])
            nc.scalar.dma_start(attn_t4, rd)

            ab_ps = P([128, 64], "pmm", bufs=4)
            nc.tensor.matmul(ab_ps, onesr, attn, start=True, stop=True)
            ab = S([128, 64], f32, "ab")
            nc.vector.tensor_copy(ab, ab_ps)

            # ---------------- enhanced ----------------
            enh_nat = []
            for b in range(B):
                if b >= 2:
                    load_mem(b, 0)
                    load_mem(b, 1)
                ep = P([128, 512], "pmm", bufs=4)
                for t in range(T):
                    if b >= 2 and t in (4, 8) :
                        load_mem(b, t // 4 + 1)
                    dg = S([128, 128], f32r, "diag", bufs=2)
                    nc.vector.tensor_scalar_mul(dg, ident,
                                                ab[:, b * T + t:b * T + t + 1])
                    nc.tensor.matmul(ep, dg, mem_slice(b, t),
                                     start=(t == 0), stop=False)
                pc_ps = P([1, 512], "pcs", bufs=2)
                nc.tensor.matmul(pc_ps, attn_t4[:, b:b + 1], pe_nat,
                                 start=True, stop=True)
                pc_sb = S([1, 512], f32r, "stage2", bufs=1)
                nc.vector.tensor_copy(pc_sb, pc_ps)
                nc.tensor.matmul(ep, onesr, pc_sb, start=False, stop=True)
                en = S([128, 512], f32, "quadA", bufs=4)
                nc.vector.tensor_copy(en, ep)
                enh_nat.append(en)

            enh_fm = [S([128, ROWS], f32r, "quadB", bufs=4) for _ in range(NCH)]
            for b in range(B):
                for c in range(NCH):
                    pt = P([128, 128], "pt", bufs=1)
                    nc.tensor.transpose(pt, enh_nat[b][:, c * 128:(c + 1) * 128],
                                        ident)
                    nc.scalar.copy(enh_fm[c][:, b * 128:(b + 1) * 128], pt)

            # fuse + output (enh@F2 + precomputed x@F1)
            f2w = load_w("f2")
            for c_out in range(NCH):
                psum = P([128, ROWS], "pmm", bufs=4)
                for c_k in range(NCH):
                    nc.tensor.matmul(
                        psum, w_chunk(f2w, c_k, c_out), enh_fm[c_k],
                        start=(c_k == 0), stop=(c_k == NCH - 1))
                fl = S([128, ROWS], f32, "feat", bufs=10)
                nc.vector.tensor_tensor(out=fl, in0=psum, in1=f1log[c_out],
                                        op=ALU.add)
                fg = S([128, ROWS], f32, "feat", bufs=10)
                nc.scalar.activation(fg, fl, AF.Sigmoid, bias=bias_col("fuse_b"))
                prod = S([128, ROWS], f32, "feat", bufs=10)
                nc.vector.tensor_tensor(out=prod, in0=fg, in1=enh_fm[c_out],
                                        op=ALU.mult)
                s1 = S([128, ROWS], f32, "feat", bufs=10)
                nc.vector.tensor_tensor(out=s1, in0=prod, in1=raw_fm[c_out],
                                        op=ALU.add)
                ofm = S([128, ROWS], f32, "feat", bufs=10)
                nc.vector.tensor_tensor(out=ofm, in0=s1, in1=x_fm[c_out],
                                        op=ALU.add)
                for b in range(B):
                    pt = P([128, 128], "pt", bufs=1)
                    nc.tensor.transpose(pt, ofm[:, b * 128:(b + 1) * 128], ident)
                    on = S([128, 128], f32, "onat", bufs=2)
                    nc.vector.tensor_copy(on, pt)
                    nc.sync.dma_start(OUT[b][:, c_out * 128:(c_out + 1) * 128], on)

    with tile.TileContext(nc) as tc:
        for _ in range(reps):
            _emit(tc)

    nc.compile()
    return nc


def _prep_maps(inputs):
    x = np.ascontiguousarray(inputs["x"], dtype=np.float32)
    mem = np.ascontiguousarray(inputs["memory_snapshot"], dtype=np.float32)

    gw = np.asarray(inputs["gate_W"], np.float32)
    fw = np.asarray(inputs["fuse_W"], np.float32)
    r = _round_f32r
    weights_r = {
        "wd": r(np.asarray(inputs["delta_W"], np.float32)),
        "wx": r(np.asarray(inputs["xproj_W"], np.float32)),
        "wpn": r(-np.asarray(inputs["phys_W"], np.float32)),
        "gx": r(gw[0:512] + gw[512:1024]),
        "gp": r(gw[1024:1536] - gw[0:512]),
        "wo": r(np.asarray(inputs["outp_W"], np.float32)),
        "f1": r(fw[0:512]),
        "f2": r(fw[512:1024]),
        "seqw": r(np.asarray(inputs["seq_W"], np.float32)),
    }
    weights_f = {
        "wm": np.asarray(inputs["mem_W"], np.float32),
        "wmd": np.asarray(inputs["memd_W"], np.float32),
        "wq": np.asarray(inputs["q_W"], np.float32),
        "wcd": np.asarray(inputs["curd_W"], np.float32),
    }
    b_t1_v = (np.asarray(inputs["xproj_b"], np.float32)
              - np.asarray(inputs["phys_b"], np.float32))
    bias_mat = np.stack([
        _bias_fm(np.asarray(inputs["delta_b"], np.float32) - b_t1_v),
        _bias_fm(b_t1_v),
        _bias_fm(np.asarray(inputs["gate_b"], np.float32)),
        _bias_fm(np.asarray(inputs["outp_b"], np.float32)),
        _bias_fm(np.asarray(inputs["q_b"], np.float32)),
        _bias_fm(np.asarray(inputs["mem_b"], np.float32)),
        _bias_fm(np.asarray(inputs["curd_b"], np.float32)),
        _bias_fm(np.asarray(inputs["memd_b"], np.float32)),
        _bias_fm(np.asarray(inputs["fuse_b"], np.float32)),
    ], axis=1).reshape(128, 36)

    sin_t = _sin_table()
    sint_dev = np.zeros((128, 64), np.float32)
    for c in range(4):
        sint_dev[:, c * 16:(c + 1) * 16] = sin_t[:, c * 128:(c + 1) * 128].T

    shared = {("W_" + n): _wdev(w) for n, w in weights_r.items()}
    shared.update({("W_" + n): _wdev(w) for n, w in weights_f.items()})
    shared.update({
        "BIAS": np.ascontiguousarray(bias_mat),
        "SEQB": r(np.asarray(inputs["seq_b"], np.float32)).reshape(1, 512),
        "SINT": r(sint_dev),
        "IDENT": np.eye(128, dtype=np.float32),
        "IDENTR": np.eye(128, dtype=np.float32),
        "ONESC": np.ones((128, 1), np.float32),
        "ONESR": np.ones((1, 128), np.float32),
        "IDENTN": -np.eye(128, dtype=np.float32),
    })

    mem_r = _round_f32r(mem)
    x_r = _round_f32r(x)
    in_maps = []
    for k in range(NC):
        sl = slice(k * LC, (k + 1) * LC)
        m = dict(shared)
        m["MEM"] = np.ascontiguousarray(mem_r[:, :, sl, :])
        m["XPH"] = np.ascontiguousarray(mem_r[:, T - 1, sl, :])
        m["XK"] = np.ascontiguousarray(x_r[:, sl, :])
        in_maps.append(m)
    return in_maps


def kernel(**inputs):
    if "nc" not in _CACHE:
        _CACHE["nc"] = _build()
    ncb = _CACHE["nc"]
    in_maps = _prep_maps(inputs)
    res = bass_utils.run_bass_kernel_spmd(ncb, in_maps, core_ids=list(range(NC)))
    out = np.empty((B, L, D), np.float32)
    for k in range(NC):
        out[:, k * LC:(k + 1) * LC, :] = res.results[k]["OUT"]
    return out
